# revision 1
# baseline (speedup 1.0000x reference)
"""Fastformer (additive attention) Bass kernel for Trainium2, 8-core data-parallel.

Math (per batch element b, derived from the reference by algebraic collapse):
    A_q   = Wq @ Wqa                                   [768, 12]   (host)
    s_q   = x @ A_q + log_mask                         [S, 12]
    e_q   = exp(s_q / 8);  den_q = sum_s e_q           [12]
    xq    = (e_q^T @ x) / (den_q + 1e-8)               [12, 768]
    q_ctx = diag-blocks of (xq @ Wq)                   [768]  (flat)
    A_k   = Wk @ (q_ctx * Wka)                         [768, 12]
    ... same pooling again -> kc0, k_ctx = q_ctx * kc0 [768]
    M     = Wq + concat_h(Wq[:,h] @ (k_ctx[h] . Wo))   [768, 768]
    out   = x @ M                                      [S, 768]

All big matmuls run in fp16 (fp32 accumulation in PSUM); the small weight-side
ops stay fp32. Sharding: batch b -> core b (B == n_cores == 8).
"""
import math
from contextlib import ExitStack

import numpy as np

import concourse.bass as bass
import concourse.bacc as bacc
import concourse.tile as tile
import concourse.mybir as mybir

F16 = mybir.dt.float16
F32 = mybir.dt.float32

B, S, F, H, D = 8, 4096, 768, 12, 64
P = 128
NF = F // P          # 6 feature chunks
NS = S // P          # 32 seq chunks of 128
NC = S // 512        # 8 seq chunks of 512


def _set_seqlen(s):
    global S, NS, NC
    S, NS, NC = s, s // P, s // 512
N_CORES = 8
EXP_SCALE = 1.0 / math.sqrt(D)   # 1/8

_prog_cache = {}


def _emit_pool_pass(nc, tc, pools, cst, A_chunks, tag_prefix):
    """Scores -> exp -> transpose -> weighted-sum pass.

    A_chunks: function j -> stationary AP [128, 12] (fp16) for feature chunk j.
    Returns (xw_psum [12,768] f32 AP, inv_den [12,1] f32 AP).
    """
    psA, psW, ework = pools["psA"], pools["psW"], pools["ework"]
    xT_sb, lm_sb, ones_sb, id_sb, x_sb = (
        cst["xT_sb"], cst["lm_sb"], cst["ones_sb"], cst["id_sb"], cst["x_sb"])

    eT = []
    for c in range(NC):
        sc = psA.tile([12, 512], F32, tag="sc")
        for j in range(NF):
            nc.tensor.matmul(sc[:], A_chunks(j), xT_sb[j][:, 512 * c:512 * (c + 1)],
                             start=(j == 0), stop=False)
        nc.tensor.matmul(sc[:], ones_sb[:], lm_sb[:, 512 * c:512 * (c + 1)],
                         start=False, stop=True)
        et = ework.tile([12, 512], F16, tag=f"eT{c}")
        nc.scalar.activation(et[:], sc[:], mybir.ActivationFunctionType.Exp,
                             scale=EXP_SCALE)
        eT.append(et)

    # transpose e to [s, 12] chunks
    e_sb = []
    for i in range(NS):
        tp = pools["psB"].tile([P, 12], F16, tag="tp")
        src = eT[i // 4][:, P * (i % 4):P * (i % 4 + 1)]
        nc.tensor.transpose(tp[:], src, id_sb[:])
        e = ework.tile([P, 12], F16, tag=f"e{i}")
        nc.vector.tensor_copy(e[:], tp[:])
        e_sb.append(e)

    xw = _emit_xw(nc, psW, x_sb, lambda i: e_sb[i][:])
    return xw


def _emit_xw(nc, psW, x_sb, e_of):
    # xw = e^T @ [x | 1] accumulated over all 32 chunks; col 768 = den
    xw = psW.tile([12, F + 1], F32, tag="wide")
    for i in range(NS):
        nc.tensor.matmul(xw[:, 0:512], e_of(i), x_sb[i][:, 0:512],
                         start=(i == 0), stop=(i == NS - 1))
        nc.tensor.matmul(xw[:, 512:F + 1], e_of(i), x_sb[i][:, 512:F + 1],
                         start=(i == 0), stop=(i == NS - 1))
    return xw


def _emit_ctx_extract(nc, tc, pools, cst, xw, W_sb, tag_prefix):
    """xw,inv -> normalized xq (f16, transposed chunks) -> G = xq @ W -> ctx col.

    Returns ctx [128, 6] f32 tile (flat [768] ctx vector, col j = f-chunk j).
    """
    ework, psW, psB = pools["ework"], pools["psW"], pools["psB"]
    id_sb = cst["id_sb"]

    inv = ework.tile([12, 1], F32, tag=f"{tag_prefix}inv")
    nc.vector.tensor_scalar_add(inv[:], xw[:, F:F + 1], 1e-8)
    nc.vector.reciprocal(inv[:], inv[:])
    xq = ework.tile([12, F], F16, tag=f"{tag_prefix}xq")
    nc.vector.tensor_scalar_mul(xq[:], xw[:, 0:F], inv[:])

    xqT = ework.tile([P, 12 * NF], F16, tag=f"{tag_prefix}xqT")
    for j in range(NF):
        tp = psB.tile([P, 12], F16, tag="tp")
        nc.tensor.transpose(tp[:], xq[:, P * j:P * (j + 1)], id_sb[:])
        nc.vector.tensor_copy(xqT[:, 12 * j:12 * (j + 1)], tp[:])

    G = psW.tile([12, F], F32, tag="wide")
    for j in range(NF):
        nc.tensor.matmul(G[:, 0:512], xqT[:, 12 * j:12 * (j + 1)],
                         W_sb[j][:, 0:512], start=(j == 0), stop=(j == NF - 1))
        nc.tensor.matmul(G[:, 512:F], xqT[:, 12 * j:12 * (j + 1)],
                         W_sb[j][:, 512:F], start=(j == 0), stop=(j == NF - 1))

    G16 = ework.tile([12, F], F16, tag=f"{tag_prefix}G16")
    nc.vector.tensor_copy(G16[:], G[:])

    ctx = ework.tile([P, NF], F32, tag=f"{tag_prefix}ctx")
    for m in range(NF):
        tp = psB.tile([P, 12], F16, tag="tp")
        nc.tensor.transpose(tp[:], G16[:, P * m:P * (m + 1)], id_sb[:])
        nc.vector.tensor_copy(ctx[0:64, m:m + 1], tp[0:64, 2 * m:2 * m + 1])
        nc.vector.tensor_copy(ctx[64:P, m:m + 1], tp[64:P, 2 * m + 1:2 * m + 2])
    return ctx


def build_program(stage=4):
    nc = bacc.Bacc(trn_type="TRN2", target_bir_lowering=False)

    xT_d = nc.dram_tensor("xT", [F, S], F16, kind="ExternalInput")
    x_d = nc.dram_tensor("x", [S, F + 1], F16, kind="ExternalInput")
    lm_d = nc.dram_tensor("lm", [1, S], F16, kind="ExternalInput")
    sqm_d = nc.dram_tensor("sqm", [P, NS * 12], F16, kind="ExternalInput")
    Wq_d = nc.dram_tensor("Wq", [F, F], F16, kind="ExternalInput")
    Wqt_d = nc.dram_tensor("Wqt", [F, F], F16, kind="ExternalInput")
    Wk_d = nc.dram_tensor("Wk", [F, F], F16, kind="ExternalInput")
    Wkt_d = nc.dram_tensor("Wkt", [F, F], F16, kind="ExternalInput")
    Wka_d = nc.dram_tensor("Wka", [F, 12], F32, kind="ExternalInput")
    Wo_d = nc.dram_tensor("Wo", [P, D], F32, kind="ExternalInput")
    id_d = nc.dram_tensor("id12", [12, 12], F16, kind="ExternalInput")
    ones_d = nc.dram_tensor("ones12", [1, 12], F16, kind="ExternalInput")
    out_d = nc.dram_tensor("out", [S, F], F32, kind="ExternalOutput")

    with tile.TileContext(nc) as tc:
        with ExitStack() as ctx:
            cpool = ctx.enter_context(tc.tile_pool(name="const", bufs=1))
            ework = ctx.enter_context(tc.tile_pool(name="ework", bufs=1))
            ost = ctx.enter_context(tc.tile_pool(name="ost", bufs=3))
            psA = ctx.enter_context(tc.tile_pool(name="psA", bufs=2, space="PSUM"))
            psB = ctx.enter_context(tc.tile_pool(name="psB", bufs=2, space="PSUM"))
            psW = ctx.enter_context(tc.tile_pool(name="psW", bufs=2, space="PSUM"))
            pools = {"psA": psA, "psB": psB, "psW": psW, "ework": ework}

            # ---- constant/resident loads
            id_sb = cpool.tile([12, 12], F16, tag="id")
            nc.sync.dma_start(id_sb[:], id_d[:])
            ones_sb = cpool.tile([1, 12], F16, tag="ones")
            nc.sync.dma_start(ones_sb[:], ones_d[:])
            lm_sb = cpool.tile([1, S], F16, tag="lm")
            nc.sync.dma_start(lm_sb[:], lm_d[:])
            sqm_sb = cpool.tile([P, NS * 12], F16, tag="sqm")
            nc.sync.dma_start(sqm_sb[:], sqm_d[:])

            xT_sb = []
            for j in range(NF):
                t = cpool.tile([P, S], F16, tag=f"xT{j}")
                nc.sync.dma_start(t[:], xT_d[P * j:P * (j + 1), :])
                xT_sb.append(t)
            x_sb = []
            for i in range(NS):
                t = cpool.tile([P, F + 1], F16, tag=f"x{i}")
                nc.sync.dma_start(t[:], x_d[P * i:P * (i + 1), :])
                x_sb.append(t)

            def load_w(dram, name):
                tiles = []
                for j in range(NF):
                    t = cpool.tile([P, F], F16, tag=f"{name}{j}")
                    nc.sync.dma_start(t[:], dram[P * j:P * (j + 1), :])
                    tiles.append(t)
                return tiles

            Wkt_sb = load_w(Wkt_d, "Wkt")
            Wk_sb = load_w(Wk_d, "Wk")
            Wq_sb = load_w(Wq_d, "Wq")
            Wqt_sb = load_w(Wqt_d, "Wqt")
            Wka_sb = cpool.tile([P, 12 * NF], F32, tag="Wka")
            for j in range(NF):
                nc.sync.dma_start(Wka_sb[:, 12 * j:12 * (j + 1)],
                                  Wka_d[P * j:P * (j + 1), :])
            Wo_sb = cpool.tile([P, D], F32, tag="Wo")
            nc.sync.dma_start(Wo_sb[:], Wo_d[:])

            cst = {"xT_sb": xT_sb, "x_sb": x_sb, "lm_sb": lm_sb,
                   "ones_sb": ones_sb, "id_sb": id_sb}

            # ---- pass 1: query pooling
            if stage >= 2:
                _build_main(nc, tc, pools, cst, cpool, ework, ost, psA, psB, psW,
                            sqm_sb, Wq_sb, Wqt_sb, Wk_sb, Wkt_sb, Wka_sb, Wo_sb,
                            xT_sb, x_sb, id_sb, out_d, stage)
            else:
                M_sb = []
                for ft in range(NF):
                    m = ework.tile([P, F], F16, tag=f"M{ft}")
                    nc.vector.tensor_copy(m[:], Wq_sb[ft][:])
                    M_sb.append(m)
                for i in range(NS):
                    ops = psW.tile([P, F], F32, tag="wide")
                    for j in range(NF):
                        lhsT = xT_sb[j][:, P * i:P * (i + 1)]
                        nc.tensor.matmul(ops[:, 0:512], lhsT, M_sb[j][:, 0:512],
                                         start=(j == 0), stop=(j == NF - 1))
                        nc.tensor.matmul(ops[:, 512:F], lhsT, M_sb[j][:, 512:F],
                                         start=(j == 0), stop=(j == NF - 1))
                    o = ost.tile([P, F], F32, tag="outst")
                    nc.vector.tensor_copy(o[:], ops[:])
                    nc.sync.dma_start(out_d[P * i:P * (i + 1), :], o[:])

    nc.compile()
    return nc


def _build_main(nc, tc, pools, cst, cpool, ework, ost, psA, psB, psW,
                sqm_sb, Wq_sb, Wqt_sb, Wk_sb, Wkt_sb, Wka_sb, Wo_sb,
                xT_sb, x_sb, id_sb, out_d, stage):
    if True:
        if True:
            eq = ework.tile([P, NS * 12], F16, tag="eq")
            nc.scalar.activation(eq[:], sqm_sb[:],
                                 mybir.ActivationFunctionType.Exp,
                                 scale=EXP_SCALE)
            xw_q = _emit_xw(nc, psW, x_sb,
                            lambda i: eq[:, 12 * i:12 * (i + 1)])
            qctx = _emit_ctx_extract(nc, tc, pools, cst, xw_q, Wq_sb, "q")

            if stage == 2:
                M_sb = []
                for ft in range(NF):
                    m = ework.tile([P, F], F16, tag=f"M{ft}")
                    nc.vector.tensor_copy(m[:], Wq_sb[ft][:])
                    M_sb.append(m)
                _emit_pass3(nc, pools, ost, xT_sb, M_sb, out_d)
                return

            # ---- A_k = Wk @ (q_ctx * Wka)
            qWka = ework.tile([P, 12 * NF], F16, tag="qWka")
            nc.vector.tensor_tensor(
                qWka[:].rearrange("p (a b) -> p a b", a=NF),
                Wka_sb[:].rearrange("p (a b) -> p a b", a=NF),
                qctx[:, :, None].broadcast_to((P, NF, 12)),
                mybir.AluOpType.mult)
            Ak_ps = psA.tile([P, 12 * NF], F32, tag="sc")
            for ft in range(NF):
                for fc in range(NF):
                    nc.tensor.matmul(
                        Ak_ps[:, 12 * ft:12 * (ft + 1)],
                        Wkt_sb[fc][:, P * ft:P * (ft + 1)],
                        qWka[:, 12 * fc:12 * (fc + 1)],
                        start=(fc == 0), stop=(fc == NF - 1))
            Ak16 = ework.tile([P, 12 * NF], F16, tag="Ak16")
            nc.vector.tensor_copy(Ak16[:], Ak_ps[:])

            # ---- pass 2: key pooling (gated)
            xw_k = _emit_pool_pass(nc, tc, pools, cst,
                                   lambda j: Ak16[:, 12 * j:12 * (j + 1)], "k")
            kc0 = _emit_ctx_extract(nc, tc, pools, cst, xw_k, Wk_sb, "k")

            if stage == 3:
                M_sb = []
                for ft in range(NF):
                    m = ework.tile([P, F], F16, tag=f"M{ft}")
                    nc.vector.tensor_copy(m[:], Wq_sb[ft][:])
                    M_sb.append(m)
                _emit_pass3(nc, pools, ost, xT_sb, M_sb, out_d)
                return

            if stage == 5:
                kctx = ework.tile([P, NF], F32, tag="kctx_prod")
                nc.vector.tensor_tensor(kctx[:], qctx[:], kc0[:],
                                        mybir.AluOpType.mult)
                TWo = []
                for j in range(NF):
                    t = ework.tile([P, D], F16, tag=f"TWo{j}")
                    nc.vector.tensor_scalar_mul(t[:], Wo_sb[:], kctx[:, j:j + 1])
                    TWo.append(t)
                M_sb = []
                for ft in range(NF):
                    m = ework.tile([P, F], F16, tag=f"M{ft}")
                    nc.vector.tensor_copy(m[:], Wq_sb[ft][:])
                    nc.vector.tensor_copy(m[:, 0:D], TWo[ft][:])
                    M_sb.append(m)
                _emit_pass3(nc, pools, ost, xT_sb, M_sb, out_d)
                return

            if stage == 6:
                Wo16 = ework.tile([P, D], F16, tag="Wo16")
                nc.vector.tensor_copy(Wo16[:], Wo_sb[:])
                M_sb = []
                for ft in range(NF):
                    Mc = psW.tile([P, F], F32, tag="wide")
                    for h in range(H):
                        lo = 64 * (h % 2)
                        nc.tensor.matmul(
                            Mc[:, D * h:D * (h + 1)],
                            Wqt_sb[h // 2][lo:lo + D, P * ft:P * (ft + 1)],
                            Wo16[lo:lo + D, :], start=True, stop=True)
                    m = ework.tile([P, F], F16, tag=f"M{ft}")
                    nc.vector.tensor_add(m[:], Mc[:], Wq_sb[ft][:])
                    M_sb.append(m)
                _emit_pass3(nc, pools, ost, xT_sb, M_sb, out_d)
                return

            kctx = ework.tile([P, NF], F32, tag="kctx_prod")
            nc.vector.tensor_tensor(kctx[:], qctx[:], kc0[:], mybir.AluOpType.mult)

            # ---- M = Wq + concat_h(Wq[:, h] @ (k_ctx[h] . Wo))
            # R[j] is a block-diagonal [128,128] gated-Wo for the head pair
            # (2j, 2j+1): rows 0:64 scale Wo by kctx head 2j into cols 0:64,
            # rows 64:128 scale (stacked) Wo by head 2j+1 into cols 64:128.
            # Keeps every matmul operand at partition base 0 (the HW rejects
            # tile_position-offset matmuls that partition-offset slices emit).
            R_sb = []
            for j in range(NF):
                r = ework.tile([P, P], F16, tag=f"R{j}")
                nc.vector.memset(r[:], 0.0)
                nc.vector.tensor_scalar_mul(r[0:64, 0:64], Wo_sb[0:64, :],
                                            kctx[0:64, j:j + 1])
                nc.vector.tensor_scalar_mul(r[64:P, 64:P], Wo_sb[64:P, :],
                                            kctx[64:P, j:j + 1])
                R_sb.append(r)

            M_sb = []
            for ft in range(NF):
                Mc = psW.tile([P, F], F32, tag="wide")
                for j in range(NF):
                    nc.tensor.matmul(Mc[:, P * j:P * (j + 1)],
                                     Wqt_sb[j][:, P * ft:P * (ft + 1)],
                                     R_sb[j][:], start=True, stop=True)
                m = ework.tile([P, F], F16, tag=f"M{ft}")
                nc.vector.tensor_add(m[:], Mc[:], Wq_sb[ft][:])
                M_sb.append(m)

            # ---- pass 3: out = x @ M
            _emit_pass3(nc, pools, ost, xT_sb, M_sb, out_d)


def _emit_pass3(nc, pools, ost, xT_sb, M_sb, out_d):
    psW = pools["psW"]
    for i in range(NS):
        ops = psW.tile([P, F], F32, tag="wide")
        for j in range(NF):
            lhsT = xT_sb[j][:, P * i:P * (i + 1)]
            nc.tensor.matmul(ops[:, 0:512], lhsT, M_sb[j][:, 0:512],
                             start=(j == 0), stop=(j == NF - 1))
            nc.tensor.matmul(ops[:, 512:F], lhsT, M_sb[j][:, 512:F],
                             start=(j == 0), stop=(j == NF - 1))
        o = ost.tile([P, F], F32, tag="outst")
        nc.vector.tensor_copy(o[:], ops[:])
        nc.sync.dma_start(out_d[P * i:P * (i + 1), :], o[:])


def _get_program():
    if "nc" not in _prog_cache:
        _prog_cache["nc"] = build_program()
    return _prog_cache["nc"]


def _prep_core_inputs(xb, maskb, w16, Aq32):
    lm = np.where(maskb > 0, np.float16(0), np.float16(-60000.0))[None, :]
    sq = xb @ Aq32                                   # [S, 12] fp32
    sq = np.where(maskb[:, None] > 0, sq, -60000.0)
    sqm = np.ascontiguousarray(
        sq.reshape(NS, P, 12).transpose(1, 0, 2).reshape(P, NS * 12))
    x1 = np.concatenate([xb, np.ones((S, 1), xb.dtype)], axis=1)
    return {
        "xT": np.ascontiguousarray(xb.T).astype(np.float16),
        "x": x1.astype(np.float16),
        "sqm": sqm.astype(np.float16),
        "lm": lm.astype(np.float16),
        **w16,
    }


def run(x, attn_mask, Wq, Wk, Wqa, Wka, Wo, trace=False):
    from concourse.bass_utils import run_bass_kernel_spmd

    nc = _get_program()
    Aq32 = Wq @ Wqa
    w16 = {
        "Wq": Wq.astype(np.float16),
        "Wqt": np.ascontiguousarray(Wq.T).astype(np.float16),
        "Wk": Wk.astype(np.float16),
        "Wkt": np.ascontiguousarray(Wk.T).astype(np.float16),
        "Wka": Wka.astype(np.float32),
        "Wo": np.vstack([Wo, Wo]).astype(np.float32),
        "id12": np.eye(12, dtype=np.float16),
        "ones12": np.ones((1, 12), dtype=np.float16),
    }
    in_maps = [_prep_core_inputs(np.asarray(x[b]), np.asarray(attn_mask[b]), w16, Aq32)
               for b in range(N_CORES)]
    res = run_bass_kernel_spmd(nc, in_maps, list(range(N_CORES)), trace=trace)
    out = np.stack([res.results[b]["out"] for b in range(N_CORES)])
    return out, res


def kernel(x, attn_mask, Wq, Wk, Wqa, Wka, Wo):
    out, _ = run(np.asarray(x, dtype=np.float32), np.asarray(attn_mask, dtype=np.float32),
                 np.asarray(Wq, dtype=np.float32), np.asarray(Wk, dtype=np.float32),
                 np.asarray(Wqa, dtype=np.float32), np.asarray(Wka, dtype=np.float32),
                 np.asarray(Wo, dtype=np.float32))
    return out



# revision 6
# speedup vs baseline: 1.7302x; 1.7302x over previous
"""Fastformer (additive attention) Bass kernel for Trainium2, 8-core data-parallel.

Math (per batch element b, algebraic collapse of the reference):
    A_q   = Wq @ Wqa                                    [768, 12]  (host weight prep)
    s_q   = x @ A_q ;  e_q = exp(s_q/8 + lm/8)          [S, 12]
    xw_q  = e_q^T @ x ; den_q = sum_s e_q               [12,768], [12]
    q_ctx = diag-blocks of ((xw_q/den_q) @ Wq)          [768]
    A_k   = Wk @ (q_ctx * Wka); same pooling -> kc0     [768]
    k_ctx = q_ctx * kc0
    M     = Wq @ (blockdiag_h(k_ctx_h * Wo) + I)        [768, 768]
    out   = x @ M                                       [S, 768]

All matmuls are oriented so outputs have tiny free dims where possible
(the pooling path), and the big x @ M pass runs as a 3-term
error-compensated fp8(e4m3) DoubleRow matmul:
    out = x8@M8 + xr8@M8 + x8@Mr8     (xr = x - x8, Mr = M - M8, PSUM x64)
Sharding: batch b -> core b (B == n_cores == 8).
"""
import math
from contextlib import ExitStack

import numpy as np
import ml_dtypes

import concourse.bass as bass
import concourse.bacc as bacc
import concourse.tile as tile
import concourse.mybir as mybir

F8 = mybir.dt.float8e4
F16 = mybir.dt.float16
F32 = mybir.dt.float32
NP8 = ml_dtypes.float8_e4m3

B, S, F, H, D = 8, 4096, 768, 12, 64
P = 128
NF = F // P            # 6 feature chunks
NS = S // P            # 32 seq chunks
GS = 4                 # seq chunks per score group
NG = NS // GS          # 8 groups
N_CORES = 8
ESC = 1.0 / math.sqrt(D)   # exp scale 1/8
MS = 64.0                  # M-side PSUM scale (power of two)
DR = mybir.MatmulPerfMode.DoubleRow

_prog_cache = {}


def _emit_pool_pass(nc, pools, cst, A3, masked):
    """One pooling pass: scores -> exp -> xw/den accumulation.

    A3: 3D AP [128, NF, 12] (fp8) score-weight chunks (rows 128j+p of A).
    Returns (xw_ps [128, NF*12+12] f32 PSUM AP with den_bcast in last 12 cols).
    """
    psS, psXW, ework = pools["psS"], pools["psXW"], pools["ework"]
    x8t3, xs8, ones128_8, lm_sb = cst["x8t3"], cst["xs8"], cst["ones128_8"], cst["lm_sb"]

    xw = psXW.tile([P, NF * 12 + 12], F32, tag="xw")
    xw3 = xw[:].rearrange("p (a b) -> p a b", a=NF + 1)
    e_tiles = []
    for g in range(NG):
        sc = psS.tile([P, GS * 12], F32, tag="sc")
        for r in range(GS):
            i = GS * g + r
            for j in range(NF):
                nc.tensor.matmul(sc[:, 12 * r:12 * (r + 1)],
                                 x8t3[:, j, P * i:P * (i + 1)], A3[:, j, :],
                                 start=(j == 0), stop=(j == NF - 1))
        e8 = ework.tile([P, GS * 12], F8, tag=f"e{g}")
        if masked:
            for r in range(GS):
                i = GS * g + r
                nc.scalar.activation(e8[:, 12 * r:12 * (r + 1)],
                                     sc[:, 12 * r:12 * (r + 1)],
                                     mybir.ActivationFunctionType.Exp,
                                     bias=lm_sb[:, i:i + 1], scale=ESC)
        else:
            nc.scalar.activation(e8[:], sc[:],
                                 mybir.ActivationFunctionType.Exp, scale=ESC)
        e_tiles.append(e8)

    for g in range(NG):
        e8 = e_tiles[g]
        for r in range(GS):
            i = GS * g + r
            first, last = (i == 0), (i == NS - 1)
            rhs = e8[:, 12 * r:12 * (r + 1)]
            for j in range(NF):
                nc.tensor.matmul(xw3[:, j, :],
                                 xs8[g][:, r, P * j:P * (j + 1)], rhs,
                                 start=first, stop=last)
            nc.tensor.matmul(xw3[:, NF, :], ones128_8[:], rhs,
                             start=first, stop=last)
    return xw


def _emit_ctx(nc, pools, cst, xw, W3, tag):
    """xw/den -> xq8 -> diagonal-head G entries -> ctx [128, NF] f32 (SBUF)."""
    psG, ework = pools["psG"], pools["ework"]
    xw3 = xw[:].rearrange("p (a b) -> p a b", a=NF + 1)

    inv = ework.tile([P, 12], F32, tag=f"inv{tag}")
    nc.vector.tensor_scalar_add(inv[:], xw[:, NF * 12:NF * 12 + 12], 1e-8)
    nc.vector.reciprocal(inv[:], inv[:])

    xq8 = ework.tile([P, NF * 12], F8, tag=f"xq{tag}")
    xq3 = xq8[:].rearrange("p (a b) -> p a b", a=NF)
    nc.vector.tensor_tensor(xq3, xw3[:, 0:NF, :],
                            inv[:, None, :].broadcast_to((P, NF, 12)),
                            mybir.AluOpType.mult)

    # only diagonal head pairs of G are needed: block m uses heads 2m, 2m+1
    gt = psG.tile([P, 2 * NF], F32, tag="g")
    for m in range(NF):
        for j in range(NF):
            nc.tensor.matmul(gt[:, 2 * m:2 * (m + 1)],
                             W3[:, j, P * m:P * (m + 1)],
                             xq3[:, j, 2 * m:2 * (m + 1)],
                             start=(j == 0), stop=(j == NF - 1))
    ctx = ework.tile([P, NF], F32, tag=f"ctx{tag}")
    gt3 = gt[:].rearrange("p (a b) -> p a b", a=NF)
    nc.vector.tensor_copy(ctx[0:64, :], gt3[0:64, :, 0])
    nc.vector.tensor_copy(ctx[64:P, :], gt3[64:P, :, 1])
    return ctx


def build_program(masked=False):
    nc = bacc.Bacc(trn_type="TRN2", target_bir_lowering=False)

    x8t_d = nc.dram_tensor("x8t", [P, NF * S], F8, kind="ExternalInput")
    xr8t_d = nc.dram_tensor("xr8t", [P, NF * S], F8, kind="ExternalInput")
    xs8_d = nc.dram_tensor("xs8", [P, NS * F], F8, kind="ExternalInput")
    aq8_d = nc.dram_tensor("aq8", [P, NF * 12], F8, kind="ExternalInput")
    wq8_d = nc.dram_tensor("wq8", [P, NF * F], F8, kind="ExternalInput")
    wk8_d = nc.dram_tensor("wk8", [P, NF * F], F8, kind="ExternalInput")
    wkt8_d = nc.dram_tensor("wkt8", [P, NF * F], F8, kind="ExternalInput")
    wqt16_d = nc.dram_tensor("wqt16", [P, NF * F], F16, kind="ExternalInput")
    wka_d = nc.dram_tensor("wka", [P, NF * 12], F32, kind="ExternalInput")
    wo64_d = nc.dram_tensor("wo64", [P, D], F32, kind="ExternalInput")
    i64_d = nc.dram_tensor("i64", [P, P], F16, kind="ExternalInput")
    ones8_d = nc.dram_tensor("ones8", [P, P], F8, kind="ExternalInput")
    lm_d = nc.dram_tensor("lm", [P, NS], F32, kind="ExternalInput")
    out_d = nc.dram_tensor("out", [S, F], F16, kind="ExternalOutput")

    with tile.TileContext(nc) as tc:
        with ExitStack() as ctx:
            cpool = ctx.enter_context(tc.tile_pool(name="const", bufs=1))
            ework = ctx.enter_context(tc.tile_pool(name="ework", bufs=1))
            obuf = ctx.enter_context(tc.tile_pool(name="obuf", bufs=2))
            psS = ctx.enter_context(tc.tile_pool(name="psS", bufs=2, space="PSUM"))
            psXW = ctx.enter_context(tc.tile_pool(name="psXW", bufs=1, space="PSUM"))
            psG = ctx.enter_context(tc.tile_pool(name="psG", bufs=1, space="PSUM"))
            psW = ctx.enter_context(tc.tile_pool(name="psW", bufs=2, space="PSUM"))
            pools = {"psS": psS, "psXW": psXW, "psG": psG, "psW": psW,
                     "ework": ework}

            # ---- loads, in consumption order
            aq8 = cpool.tile([P, NF * 12], F8, tag="aq8")
            nc.sync.dma_start(aq8[:], aq8_d[:])
            ones128_8 = cpool.tile([P, P], F8, tag="ones8")
            nc.sync.dma_start(ones128_8[:], ones8_d[:])
            lm_sb = cpool.tile([P, NS], F32, tag="lm")
            if masked:
                nc.sync.dma_start(lm_sb[:], lm_d[:])
            x8t = cpool.tile([P, NF * S], F8, tag="x8t")
            nc.sync.dma_start(x8t[:], x8t_d[:])
            xs8 = []
            for g in range(NG):
                t = cpool.tile([P, GS * F], F8, tag=f"xs8_{g}")
                nc.sync.dma_start(t[:], xs8_d[:, GS * F * g:GS * F * (g + 1)])
                xs8.append(t[:].rearrange("p (a b) -> p a b", a=GS))
            wq8 = cpool.tile([P, NF * F], F8, tag="wq8")
            nc.sync.dma_start(wq8[:], wq8_d[:])
            wkt8 = cpool.tile([P, NF * F], F8, tag="wkt8")
            nc.sync.dma_start(wkt8[:], wkt8_d[:])
            wk8 = cpool.tile([P, NF * F], F8, tag="wk8")
            nc.sync.dma_start(wk8[:], wk8_d[:])
            wka = cpool.tile([P, NF * 12], F32, tag="wka")
            nc.sync.dma_start(wka[:], wka_d[:])
            wo64 = cpool.tile([P, D], F32, tag="wo64")
            nc.sync.dma_start(wo64[:], wo64_d[:])
            wqt16 = cpool.tile([P, NF * F], F16, tag="wqt16")
            nc.sync.dma_start(wqt16[:], wqt16_d[:])
            i64 = cpool.tile([P, P], F16, tag="i64")
            nc.sync.dma_start(i64[:], i64_d[:])
            xr8t = cpool.tile([P, NF * S], F8, tag="xr8t")
            nc.sync.dma_start(xr8t[:], xr8t_d[:])

            x8t3 = x8t[:].rearrange("p (a b) -> p a b", a=NF)
            xr8t3 = xr8t[:].rearrange("p (a b) -> p a b", a=NF)
            wq3 = wq8[:].rearrange("p (a b) -> p a b", a=NF)
            wk3 = wk8[:].rearrange("p (a b) -> p a b", a=NF)
            wkt3 = wkt8[:].rearrange("p (a b) -> p a b", a=NF)
            wqt3 = wqt16[:].rearrange("p (a b) -> p a b", a=NF)
            wka3 = wka[:].rearrange("p (a b) -> p a b", a=NF)
            aq3 = aq8[:].rearrange("p (a b) -> p a b", a=NF)
            cst = {"x8t3": x8t3, "xs8": xs8, "ones128_8": ones128_8,
                   "lm_sb": lm_sb}

            # ---- pass 1: query pooling + q_ctx
            xw_q = _emit_pool_pass(nc, pools, cst, aq3, masked)
            qctx = _emit_ctx(nc, pools, cst, xw_q, wq3, "q")

            # ---- A_k = Wk @ (q_ctx * Wka)
            g8 = ework.tile([P, NF * 12], F8, tag="g8")
            g3 = g8[:].rearrange("p (a b) -> p a b", a=NF)
            nc.vector.tensor_tensor(g3, wka3,
                                    qctx[:, :, None].broadcast_to((P, NF, 12)),
                                    mybir.AluOpType.mult)
            ak_ps = psG.tile([P, NF * 12], F32, tag="g")
            for ft in range(NF):
                for fc in range(NF):
                    nc.tensor.matmul(ak_ps[:, 12 * ft:12 * (ft + 1)],
                                     wkt3[:, fc, P * ft:P * (ft + 1)],
                                     g3[:, fc, :],
                                     start=(fc == 0), stop=(fc == NF - 1))
            ak8 = ework.tile([P, NF * 12], F8, tag="ak8")
            nc.scalar.copy(ak8[:], ak_ps[:])
            ak3 = ak8[:].rearrange("p (a b) -> p a b", a=NF)

            # ---- pass 2: key pooling + k_ctx
            xw_k = _emit_pool_pass(nc, pools, cst, ak3, masked)
            kc0 = _emit_ctx(nc, pools, cst, xw_k, wk3, "k")
            kctx = ework.tile([P, NF], F32, tag="kctx")
            nc.vector.tensor_tensor(kctx[:], qctx[:], kc0[:],
                                    mybir.AluOpType.mult)

            # ---- M = Wq @ (blockdiag(kctx_h * Wo) + I), scaled by MS
            # R_j holds the gated-Wo blocks for head pair (2j, 2j+1);
            # the +I (i.e. + Wq) lands via a second matmul against MS*I128.
            R_sb = []
            for j in range(NF):
                r = ework.tile([P, P], F16, tag=f"R{j}")
                nc.vector.memset(r[:], 0.0)
                nc.vector.tensor_scalar_mul(r[0:64, 0:64], wo64[0:64, :],
                                            kctx[0:64, j:j + 1])
                nc.vector.tensor_scalar_mul(r[64:P, 64:P], wo64[64:P, :],
                                            kctx[64:P, j:j + 1])
                R_sb.append(r)

            m8 = ework.tile([P, NF * F], F8, tag="m8")
            mr8 = ework.tile([P, NF * F], F8, tag="mr8")
            m8_3 = m8[:].rearrange("p (a b) -> p a b", a=NF)
            mr8_3 = mr8[:].rearrange("p (a b) -> p a b", a=NF)
            for ft in range(NF):
                mc = psW.tile([P, F], F32, tag="wide")
                for j in range(NF):
                    reg = mc[:, P * j:P * (j + 1)]
                    lhsT = wqt3[:, j, P * ft:P * (ft + 1)]
                    nc.tensor.matmul(reg, lhsT, R_sb[j][:], start=True, stop=False)
                    nc.tensor.matmul(reg, lhsT, i64[:], start=False, stop=True)
                nc.scalar.copy(m8_3[:, ft, :], mc[:])
                nc.vector.tensor_tensor(mr8_3[:, ft, :], mc[:], m8_3[:, ft, :],
                                        mybir.AluOpType.subtract)

            # ---- pass 3: out = (x8 + xr8) @ M8 + x8 @ Mr8, fp8 DoubleRow
            for g in range(NG):
                ow = obuf.tile([P, GS * F], F16, tag="ow")
                for r in range(GS):
                    i = GS * g + r
                    ps = psW.tile([P, F], F32, tag="wide")
                    for lo, hi in ((0, 512), (512, F)):
                        n = 0
                        for lhs3, rhs3 in ((x8t3, m8_3), (x8t3, mr8_3),
                                           (xr8t3, m8_3)):
                            for t in range(NF // 2):
                                nc.tensor.matmul(
                                    ps[:, lo:hi],
                                    lhs3[:, 2 * t:2 * t + 2, P * i:P * (i + 1)],
                                    rhs3[:, 2 * t:2 * t + 2, lo:hi],
                                    start=(n == 0), stop=(n == 8),
                                    perf_mode=DR)
                                n += 1
                    nc.scalar.mul(ow[:, F * r:F * (r + 1)], ps[:], 1.0 / MS)
                nc.sync.dma_start(
                    out_d[GS * P * g:GS * P * (g + 1), :]
                    .rearrange("(a p) b -> p a b", p=P),
                    ow[:].rearrange("p (a b) -> p a b", a=GS))

    nc.compile()
    return nc


def _get_program(masked=False):
    key = ("m" if masked else "u")
    if key not in _prog_cache:
        _prog_cache[key] = build_program(masked)
    return _prog_cache[key]


def _chunk_rows(a, np_dtype):
    """[R*128, C] -> [128, R*C] with chunk r of rows at cols [r*C:(r+1)*C]."""
    R = a.shape[0] // P
    return np.ascontiguousarray(
        a.reshape(R, P, a.shape[1]).transpose(1, 0, 2).reshape(P, -1)
    ).astype(np_dtype)


def _prep_weights(Wq, Wk, Wqa, Wka, Wo):
    Aq = (Wq @ Wqa).astype(np.float32)
    return {
        "aq8": _chunk_rows(Aq, NP8),
        "wq8": _chunk_rows(Wq, NP8),
        "wk8": _chunk_rows(Wk, NP8),
        "wkt8": _chunk_rows(np.ascontiguousarray(Wk.T), NP8),
        "wqt16": _chunk_rows(np.ascontiguousarray(Wq.T), np.float16),
        "wka": _chunk_rows(Wka, np.float32),
        "wo64": (MS * np.vstack([Wo, Wo])).astype(np.float32),
        "i64": (MS * np.eye(P)).astype(np.float16),
        "ones8": np.ones((P, P), NP8),
    }


def _prep_core_inputs(xb, maskb, w, masked):
    x8 = xb.astype(NP8)
    xr8 = (xb - x8.astype(np.float32)).astype(NP8)
    d = {
        "x8t": _chunk_rows(np.ascontiguousarray(x8.astype(np.float32).T), NP8),
        "xr8t": _chunk_rows(np.ascontiguousarray(xr8.astype(np.float32).T), NP8),
        "xs8": _chunk_rows(x8.astype(np.float32), NP8),
        "lm": np.zeros((P, NS), np.float32),
        **w,
    }
    if masked:
        lm = np.where(maskb > 0, 0.0, -60000.0).astype(np.float32) * ESC
        d["lm"] = np.ascontiguousarray(lm.reshape(NS, P).T)
    return d


def run(x, attn_mask, Wq, Wk, Wqa, Wka, Wo, trace=False):
    from concourse.bass_utils import run_bass_kernel_spmd

    masked = not bool(np.all(attn_mask == 1.0))
    nc = _get_program(masked)
    w = _prep_weights(Wq, Wk, Wqa, Wka, Wo)
    in_maps = [_prep_core_inputs(np.asarray(x[b]), np.asarray(attn_mask[b]),
                                 w, masked)
               for b in range(N_CORES)]
    res = run_bass_kernel_spmd(nc, in_maps, list(range(N_CORES)), trace=trace)
    out = np.stack([res.results[b]["out"].astype(np.float32)
                    for b in range(N_CORES)])
    return out, res


def kernel(x, attn_mask, Wq, Wk, Wqa, Wka, Wo):
    out, _ = run(np.asarray(x, dtype=np.float32),
                 np.asarray(attn_mask, dtype=np.float32),
                 np.asarray(Wq, dtype=np.float32),
                 np.asarray(Wk, dtype=np.float32),
                 np.asarray(Wqa, dtype=np.float32),
                 np.asarray(Wka, dtype=np.float32),
                 np.asarray(Wo, dtype=np.float32))
    return out


# revision 7
# speedup vs baseline: 1.8197x; 1.0517x over previous
"""Fastformer (additive attention) Bass kernel for Trainium2, 8-core data-parallel.

Math (per batch element b, algebraic collapse of the reference):
    A_q   = Wq @ Wqa                                    [768, 12]  (host weight prep)
    s_q   = x @ A_q ;  e_q = exp(s_q/8 + lm/8)          [S, 12]
    xw_q  = e_q^T @ x ; den_q = sum_s e_q               [12,768], [12]
    q_ctx = diag-blocks of ((xw_q/den_q) @ Wq)          [768]
    A_k   = Wk @ (q_ctx * Wka); same pooling -> kc0     [768]
    k_ctx = q_ctx * kc0
    M     = Wq @ (blockdiag_h(k_ctx_h * Wo) + I)        [768, 768]
    out   = x @ M                                       [S, 768]

Pooling-path matmuls are oriented so outputs have tiny free dims; the big
x @ M pass runs as a 3-term error-compensated fp8(e4m3) DoubleRow matmul:
    out = x8@M8 + x8@Mr8 + xr8@M8     (xr = x - x8, Mr = M - M8, PSUM x64)
Sharding: batch b -> core b (B == n_cores == 8).
"""
import math
from contextlib import ExitStack

import numpy as np
import ml_dtypes

import concourse.bass as bass
import concourse.bacc as bacc
import concourse.tile as tile
import concourse.mybir as mybir

F8 = mybir.dt.float8e4
F16 = mybir.dt.float16
F32 = mybir.dt.float32
NP8 = ml_dtypes.float8_e4m3

B, S, F, H, D = 8, 4096, 768, 12, 64
P = 128
NF = F // P            # 6 feature chunks
NS = S // P            # 32 seq chunks
GS = 4                 # seq chunks per score group
NG = NS // GS          # 8 groups
N_CORES = 8
ESC = 1.0 / math.sqrt(D)   # exp scale 1/8
MS = 64.0                  # M-side PSUM scale (power of two)
DR = mybir.MatmulPerfMode.DoubleRow

_prog_cache = {}


def _emit_scores(nc, pools, cst, A3, masked, half, e_tiles):
    """Scores + exp for groups covered by x8t column half `half`."""
    psS, ework = pools["psS"], pools["ework"]
    x8t3, lm_sb = cst["x8t3"], cst["lm_sb"]
    for g in range(NG // 2 * half, NG // 2 * (half + 1)):
        sc = psS.tile([P, GS * 12], F32, tag="sc")
        for r in range(GS):
            i = GS * g + r
            for j in range(NF):
                nc.tensor.matmul(sc[:, 12 * r:12 * (r + 1)],
                                 x8t3[:, j, P * i:P * (i + 1)], A3[:, j, :],
                                 start=(j == 0), stop=(j == NF - 1))
        e8 = ework.tile([P, GS * 12], F8, tag=f"e{g}")
        if masked:
            for r in range(GS):
                i = GS * g + r
                nc.scalar.activation(e8[:, 12 * r:12 * (r + 1)],
                                     sc[:, 12 * r:12 * (r + 1)],
                                     mybir.ActivationFunctionType.Exp,
                                     bias=lm_sb[:, i:i + 1], scale=ESC)
        else:
            nc.scalar.activation(e8[:], sc[:],
                                 mybir.ActivationFunctionType.Exp, scale=ESC)
        e_tiles.append(e8)


def _emit_xw(nc, pools, cst, e_tiles, groups, xw, first_i, last_i):
    """Accumulate xw/den over the given groups into xw (PSUM [P,(NF+1)*12])."""
    ones128_8, xs8 = cst["ones128_8"], cst["xs8"]
    xw3 = xw[:].rearrange("p (a b) -> p a b", a=NF + 1)
    for g in groups:
        e8 = e_tiles[g]
        for r in range(GS):
            i = GS * g + r
            first, last = (i == first_i), (i == last_i)
            rhs = e8[:, 12 * r:12 * (r + 1)]
            for j in range(NF):
                nc.tensor.matmul(xw3[:, j, :],
                                 xs8[g][:, r, P * j:P * (j + 1)], rhs,
                                 start=first, stop=last)
            nc.tensor.matmul(xw3[:, NF, :], ones128_8[:], rhs,
                             start=first, stop=last)


def _emit_ctx(nc, pools, xw, W3, tag):
    """xw/den -> xq8 -> diagonal-head G entries -> ctx [128, NF] f32 (SBUF)."""
    psG, ework = pools["psG"], pools["ework"]
    xw3 = xw[:].rearrange("p (a b) -> p a b", a=NF + 1)

    inv = ework.tile([P, 12], F32, tag=f"inv{tag}")
    nc.vector.tensor_scalar_add(inv[:], xw[:, NF * 12:NF * 12 + 12], 1e-8)
    nc.vector.reciprocal(inv[:], inv[:])

    xq8 = ework.tile([P, NF * 12], F8, tag=f"xq{tag}")
    xq3 = xq8[:].rearrange("p (a b) -> p a b", a=NF)
    nc.vector.tensor_tensor(xq3, xw3[:, 0:NF, :],
                            inv[:, None, :].broadcast_to((P, NF, 12)),
                            mybir.AluOpType.mult)

    # only diagonal head pairs of G are needed: block m uses heads 2m, 2m+1
    gt = psG.tile([P, 2 * NF], F32, tag="g")
    for m in range(NF):
        for j in range(NF):
            nc.tensor.matmul(gt[:, 2 * m:2 * (m + 1)],
                             W3[:, j, P * m:P * (m + 1)],
                             xq3[:, j, 2 * m:2 * (m + 1)],
                             start=(j == 0), stop=(j == NF - 1))
    ctx = ework.tile([P, NF], F32, tag=f"ctx{tag}")
    gt3 = gt[:].rearrange("p (a b) -> p a b", a=NF)
    nc.vector.tensor_copy(ctx[0:64, :], gt3[0:64, :, 0])
    nc.vector.tensor_copy(ctx[64:P, :], gt3[64:P, :, 1])
    return ctx


def build_program(masked=False):
    nc = bacc.Bacc(trn_type="TRN2", target_bir_lowering=False)

    x8t_d = nc.dram_tensor("x8t", [P, NF * S], F8, kind="ExternalInput")
    xr8t_d = nc.dram_tensor("xr8t", [P, NF * S], F8, kind="ExternalInput")
    xs8_d = nc.dram_tensor("xs8", [P, NS * F], F8, kind="ExternalInput")
    aq8_d = nc.dram_tensor("aq8", [P, NF * 12], F8, kind="ExternalInput")
    wq8_d = nc.dram_tensor("wq8", [P, NF * F], F8, kind="ExternalInput")
    wk8_d = nc.dram_tensor("wk8", [P, NF * F], F8, kind="ExternalInput")
    wkt8_d = nc.dram_tensor("wkt8", [P, NF * F], F8, kind="ExternalInput")
    wqt16_d = nc.dram_tensor("wqt16", [P, NF * F], F16, kind="ExternalInput")
    wka_d = nc.dram_tensor("wka", [P, NF * 12], F32, kind="ExternalInput")
    wobd_d = nc.dram_tensor("wobd", [P, P], F32, kind="ExternalInput")
    i64_d = nc.dram_tensor("i64", [P, P], F16, kind="ExternalInput")
    ones8_d = nc.dram_tensor("ones8", [P, P], F8, kind="ExternalInput")
    lm_d = nc.dram_tensor("lm", [P, NS], F32, kind="ExternalInput")
    out_d = nc.dram_tensor("out", [S, F], F16, kind="ExternalOutput")

    with tile.TileContext(nc) as tc:
        with ExitStack() as ctx:
            cpool = ctx.enter_context(tc.tile_pool(name="const", bufs=1))
            ework = ctx.enter_context(tc.tile_pool(name="ework", bufs=1))
            obuf = ctx.enter_context(tc.tile_pool(name="obuf", bufs=3))
            psW = ctx.enter_context(tc.tile_pool(name="psW", bufs=2, space="PSUM"))

            # ---- loads, in consumption order; small tensors first
            aq8 = cpool.tile([P, NF * 12], F8, tag="aq8")
            nc.sync.dma_start(aq8[:], aq8_d[:])
            ones128_8 = cpool.tile([P, P], F8, tag="ones8")
            nc.sync.dma_start(ones128_8[:], ones8_d[:])
            lm_sb = cpool.tile([P, NS], F32, tag="lm")
            if masked:
                nc.sync.dma_start(lm_sb[:], lm_d[:])
            wka = cpool.tile([P, NF * 12], F32, tag="wka")
            nc.sync.dma_start(wka[:], wka_d[:])
            wobd = cpool.tile([P, P], F32, tag="wobd")
            nc.sync.dma_start(wobd[:], wobd_d[:])
            i64 = cpool.tile([P, P], F16, tag="i64")
            nc.sync.dma_start(i64[:], i64_d[:])

            x8t = cpool.tile([P, NF * S], F8, tag="x8t")
            x8t3 = x8t[:].rearrange("p (a b) -> p a b", a=NF)
            x8t_d3 = x8t_d[:].rearrange("p (a b) -> p a b", a=NF)
            xs8_tiles = []
            xs8 = []
            for g in range(NG):
                t = cpool.tile([P, GS * F], F8, tag=f"xs8_{g}")
                xs8_tiles.append(t)
                xs8.append(t[:].rearrange("p (a b) -> p a b", a=GS))
            # interleave: xT half 1, xs groups 0-3, xT half 2, xs groups 4-7
            nc.sync.dma_start(x8t3[:, :, 0:S // 2], x8t_d3[:, :, 0:S // 2])
            for g in range(NG // 2):
                nc.sync.dma_start(xs8_tiles[g][:],
                                  xs8_d[:, GS * F * g:GS * F * (g + 1)])
            nc.sync.dma_start(x8t3[:, :, S // 2:S], x8t_d3[:, :, S // 2:S])
            for g in range(NG // 2, NG):
                nc.sync.dma_start(xs8_tiles[g][:],
                                  xs8_d[:, GS * F * g:GS * F * (g + 1)])

            wq8 = cpool.tile([P, NF * F], F8, tag="wq8")
            nc.sync.dma_start(wq8[:], wq8_d[:])
            wkt8 = cpool.tile([P, NF * F], F8, tag="wkt8")
            nc.sync.dma_start(wkt8[:], wkt8_d[:])
            wk8 = cpool.tile([P, NF * F], F8, tag="wk8")
            nc.sync.dma_start(wk8[:], wk8_d[:])
            wqt16 = cpool.tile([P, NF * F], F16, tag="wqt16")
            nc.sync.dma_start(wqt16[:], wqt16_d[:])
            xr8t = cpool.tile([P, NF * S], F8, tag="xr8t")
            xr8t3 = xr8t[:].rearrange("p (a b) -> p a b", a=NF)
            xr8t_d3 = xr8t_d[:].rearrange("p (a b) -> p a b", a=NF)
            for q in range(4):
                lo, hi = S // 4 * q, S // 4 * (q + 1)
                nc.sync.dma_start(xr8t3[:, :, lo:hi], xr8t_d3[:, :, lo:hi])

            wq3 = wq8[:].rearrange("p (a b) -> p a b", a=NF)
            wk3 = wk8[:].rearrange("p (a b) -> p a b", a=NF)
            wkt3 = wkt8[:].rearrange("p (a b) -> p a b", a=NF)
            wqt3 = wqt16[:].rearrange("p (a b) -> p a b", a=NF)
            wka3 = wka[:].rearrange("p (a b) -> p a b", a=NF)
            aq3 = aq8[:].rearrange("p (a b) -> p a b", a=NF)
            cst = {"x8t3": x8t3, "xs8": xs8, "ones128_8": ones128_8,
                   "lm_sb": lm_sb}

            m8 = ework.tile([P, NF * F], F8, tag="m8")
            mr8 = ework.tile([P, NF * F], F8, tag="mr8")
            m8_3 = m8[:].rearrange("p (a b) -> p a b", a=NF)
            mr8_3 = mr8[:].rearrange("p (a b) -> p a b", a=NF)

            with ExitStack() as pre:
                psS = pre.enter_context(tc.tile_pool(name="psS", bufs=2,
                                                     space="PSUM"))
                psXW = pre.enter_context(tc.tile_pool(name="psXW", bufs=1,
                                                      space="PSUM"))
                psG = pre.enter_context(tc.tile_pool(name="psG", bufs=1,
                                                     space="PSUM"))
                pools = {"psS": psS, "psXW": psXW, "psG": psG, "ework": ework}

                # ---- pass 1: query pooling + q_ctx (split by xT halves)
                e_q = []
                xw_q = psXW.tile([P, (NF + 1) * 12], F32, tag="xw")
                _emit_scores(nc, pools, cst, aq3, masked, 0, e_q)
                _emit_xw(nc, pools, cst, e_q, range(NG // 2), xw_q, 0, NS - 1)
                _emit_scores(nc, pools, cst, aq3, masked, 1, e_q)
                _emit_xw(nc, pools, cst, e_q, range(NG // 2, NG), xw_q,
                         0, NS - 1)
                qctx = _emit_ctx(nc, pools, xw_q, wq3, "q")

                # ---- A_k = Wk @ (q_ctx * Wka)
                g8 = ework.tile([P, NF * 12], F8, tag="g8")
                g3 = g8[:].rearrange("p (a b) -> p a b", a=NF)
                nc.vector.tensor_tensor(
                    g3, wka3, qctx[:, :, None].broadcast_to((P, NF, 12)),
                    mybir.AluOpType.mult)
                ak_ps = psG.tile([P, NF * 12], F32, tag="g")
                for ft in range(NF):
                    for fc in range(NF):
                        nc.tensor.matmul(ak_ps[:, 12 * ft:12 * (ft + 1)],
                                         wkt3[:, fc, P * ft:P * (ft + 1)],
                                         g3[:, fc, :],
                                         start=(fc == 0), stop=(fc == NF - 1))
                ak8 = ework.tile([P, NF * 12], F8, tag="ak8")
                nc.scalar.copy(ak8[:], ak_ps[:])
                ak3 = ak8[:].rearrange("p (a b) -> p a b", a=NF)

                # ---- pass 2: key pooling + k_ctx
                e_k = []
                xw_k = psXW.tile([P, (NF + 1) * 12], F32, tag="xw")
                _emit_scores(nc, pools, cst, ak3, masked, 0, e_k)
                _emit_scores(nc, pools, cst, ak3, masked, 1, e_k)
                _emit_xw(nc, pools, cst, e_k, range(NG), xw_k, 0, NS - 1)
                kc0 = _emit_ctx(nc, pools, xw_k, wk3, "k")
                kctx = ework.tile([P, NF], F32, tag="kctx")
                nc.vector.tensor_tensor(kctx[:], qctx[:], kc0[:],
                                        mybir.AluOpType.mult)

                # ---- M = Wq @ (blockdiag(kctx_h * Wo) + I), scaled by MS
                # r_all[:, j, :] = wobd (block-diag stacked Wo, x64) row-scaled
                # by kctx[:, j]; the +I (i.e. + Wq) lands via MS*I128 matmuls.
                r_all = ework.tile([P, NF * P], F16, tag="r_all")
                r3 = r_all[:].rearrange("p (a b) -> p a b", a=NF)
                nc.vector.tensor_tensor(
                    r3, wobd[:, None, :].broadcast_to((P, NF, P)),
                    kctx[:, :, None].broadcast_to((P, NF, P)),
                    mybir.AluOpType.mult)

                for ft in range(NF):
                    mc = psW.tile([P, F], F32, tag="wide")
                    for j in range(NF):
                        reg = mc[:, P * j:P * (j + 1)]
                        lhsT = wqt3[:, j, P * ft:P * (ft + 1)]
                        nc.tensor.matmul(reg, lhsT, r3[:, j, :],
                                         start=True, stop=False)
                        nc.tensor.matmul(reg, lhsT, i64[:],
                                         start=False, stop=True)
                    nc.scalar.copy(m8_3[:, ft, :], mc[:])
                    nc.vector.tensor_tensor(mr8_3[:, ft, :], mc[:],
                                            m8_3[:, ft, :],
                                            mybir.AluOpType.subtract)

            # ---- pass 3: out = (x8 + xr8) @ M8 + x8 @ Mr8, fp8 DoubleRow,
            # pair-major accumulation; two pools alternate -> 4 chunks in flight
            psT = ctx.enter_context(tc.tile_pool(name="psT", bufs=2,
                                                 space="PSUM"))
            for i in range(NS):
                ps = (psW if i % 2 == 0 else psT).tile([P, F], F32, tag="wide")
                n = 0
                for t in range(NF // 2):
                    for lhs3, rhs3 in ((x8t3, m8_3), (x8t3, mr8_3),
                                      (xr8t3, m8_3)):
                        for lo, hi in ((0, 512), (512, F)):
                            nc.tensor.matmul(
                                ps[:, lo:hi],
                                lhs3[:, 2 * t:2 * t + 2, P * i:P * (i + 1)],
                                rhs3[:, 2 * t:2 * t + 2, lo:hi],
                                start=(n == 0), stop=(n == 16),
                                perf_mode=DR)
                        n += 2
                ow = obuf.tile([P, F], F16, tag="ow")
                nc.scalar.mul(ow[:], ps[:], 1.0 / MS)
                nc.sync.dma_start(out_d[P * i:P * (i + 1), :], ow[:])

    nc.compile()
    return nc


def _get_program(masked=False):
    key = ("m" if masked else "u")
    if key not in _prog_cache:
        _prog_cache[key] = build_program(masked)
    return _prog_cache[key]


def _chunk_rows(a, np_dtype):
    """[R*128, C] -> [128, R*C] with chunk r of rows at cols [r*C:(r+1)*C]."""
    R = a.shape[0] // P
    return np.ascontiguousarray(
        a.reshape(R, P, a.shape[1]).transpose(1, 0, 2).reshape(P, -1)
    ).astype(np_dtype)


def _prep_weights(Wq, Wk, Wqa, Wka, Wo):
    Aq = (Wq @ Wqa).astype(np.float32)
    wobd = np.zeros((P, P), np.float32)
    wobd[0:64, 0:64] = MS * Wo
    wobd[64:P, 64:P] = MS * Wo
    return {
        "aq8": _chunk_rows(Aq, NP8),
        "wq8": _chunk_rows(Wq, NP8),
        "wk8": _chunk_rows(Wk, NP8),
        "wkt8": _chunk_rows(np.ascontiguousarray(Wk.T), NP8),
        "wqt16": _chunk_rows(np.ascontiguousarray(Wq.T), np.float16),
        "wka": _chunk_rows(Wka, np.float32),
        "wobd": wobd,
        "i64": (MS * np.eye(P)).astype(np.float16),
        "ones8": np.ones((P, P), NP8),
    }


def _prep_core_inputs(xb, maskb, w, masked):
    x8 = xb.astype(NP8)
    xr8 = (xb - x8.astype(np.float32)).astype(NP8)
    d = {
        "x8t": _chunk_rows(np.ascontiguousarray(x8.astype(np.float32).T), NP8),
        "xr8t": _chunk_rows(np.ascontiguousarray(xr8.astype(np.float32).T), NP8),
        "xs8": _chunk_rows(x8.astype(np.float32), NP8),
        "lm": np.zeros((P, NS), np.float32),
        **w,
    }
    if masked:
        lm = np.where(maskb > 0, 0.0, -60000.0).astype(np.float32) * ESC
        d["lm"] = np.ascontiguousarray(lm.reshape(NS, P).T)
    return d


def run(x, attn_mask, Wq, Wk, Wqa, Wka, Wo, trace=False):
    from concourse.bass_utils import run_bass_kernel_spmd

    masked = not bool(np.all(attn_mask == 1.0))
    nc = _get_program(masked)
    w = _prep_weights(Wq, Wk, Wqa, Wka, Wo)
    in_maps = [_prep_core_inputs(np.asarray(x[b]), np.asarray(attn_mask[b]),
                                 w, masked)
               for b in range(N_CORES)]
    res = run_bass_kernel_spmd(nc, in_maps, list(range(N_CORES)), trace=trace)
    out = np.stack([res.results[b]["out"].astype(np.float32)
                    for b in range(N_CORES)])
    return out, res


def kernel(x, attn_mask, Wq, Wk, Wqa, Wka, Wo):
    out, _ = run(np.asarray(x, dtype=np.float32),
                 np.asarray(attn_mask, dtype=np.float32),
                 np.asarray(Wq, dtype=np.float32),
                 np.asarray(Wk, dtype=np.float32),
                 np.asarray(Wqa, dtype=np.float32),
                 np.asarray(Wka, dtype=np.float32),
                 np.asarray(Wo, dtype=np.float32))
    return out


# revision 10
# speedup vs baseline: 1.9026x; 1.0456x over previous
"""Fastformer (additive attention) Bass kernel for Trainium2, 8-core data-parallel.

Math (per batch element b, algebraic collapse of the reference):
    A_q   = Wq @ Wqa                                    [768, 12]  (host weight prep)
    s_q   = x @ A_q ;  e_q = exp(s_q/8 + lm/8)          [S, 12]
    xw_q  = e_q^T @ x ; den_q = sum_s e_q               [12,768], [12]
    q_ctx = diag-blocks of ((xw_q/den_q) @ Wq)          [768]
    A_k   = Wk @ (q_ctx * Wka); same pooling -> kc0     [768]
    k_ctx = q_ctx * kc0
    M     = Wq @ (blockdiag_h(k_ctx_h * Wo) + I)        [768, 768]
    out   = x @ M                                       [S, 768]

Pooling-path matmuls are oriented so outputs have tiny free dims; the big
x @ M pass runs as a 3-term error-compensated fp8(e4m3) DoubleRow matmul:
    out = x8@M8 + x8@Mr8 + xr8@M8     (xr = x - x8, Mr = M - M8, PSUM x64)
Sharding: batch b -> core b (B == n_cores == 8).
"""
import math
from contextlib import ExitStack

import numpy as np
import ml_dtypes

import concourse.bass as bass
import concourse.bacc as bacc
import concourse.tile as tile
import concourse.mybir as mybir

F8 = mybir.dt.float8e4
F16 = mybir.dt.float16
F32 = mybir.dt.float32
NP8 = ml_dtypes.float8_e4m3

B, S, F, H, D = 8, 4096, 768, 12, 64
P = 128
NF = F // P            # 6 feature chunks
NS = S // P            # 32 seq chunks
GS = 4                 # seq chunks per score group
NG = NS // GS          # 8 groups
N_CORES = 8
ESC = 1.0 / math.sqrt(D)   # exp scale 1/8
MS = 64.0                  # M-side PSUM scale (power of two)
DR = mybir.MatmulPerfMode.DoubleRow

_prog_cache = {}


def _emit_scores(nc, pools, cst, A3, masked, half, e_tiles):
    """Scores + exp for groups covered by x8t column half `half`."""
    psS, ework = pools["psS"], pools["ework"]
    x8t3, lm_sb = cst["x8t3"], cst["lm_sb"]
    for g in range(NG // 2 * half, NG // 2 * (half + 1)):
        sc = psS.tile([P, GS * 12], F32, tag="sc")
        for r in range(GS):
            i = GS * g + r
            for j in range(NF):
                nc.tensor.matmul(sc[:, 12 * r:12 * (r + 1)],
                                 x8t3[:, j, P * i:P * (i + 1)], A3[:, j, :],
                                 start=(j == 0), stop=(j == NF - 1))
        e8 = ework.tile([P, GS * 12], F8, tag=f"e{g}")
        if masked:
            for r in range(GS):
                i = GS * g + r
                nc.scalar.activation(e8[:, 12 * r:12 * (r + 1)],
                                     sc[:, 12 * r:12 * (r + 1)],
                                     mybir.ActivationFunctionType.Exp,
                                     bias=lm_sb[:, i:i + 1], scale=ESC)
        else:
            nc.scalar.activation(e8[:], sc[:],
                                 mybir.ActivationFunctionType.Exp, scale=ESC)
        e_tiles.append(e8)


def _emit_xw(nc, pools, cst, e_tiles, groups, xw, first_i, last_i):
    """Accumulate xw/den over the given groups into xw (PSUM [P,(NF+1)*12])."""
    ones128_8, xs8 = cst["ones128_8"], cst["xs8"]
    xw3 = xw[:].rearrange("p (a b) -> p a b", a=NF + 1)
    for g in groups:
        e8 = e_tiles[g]
        for r in range(GS):
            i = GS * g + r
            first, last = (i == first_i), (i == last_i)
            rhs = e8[:, 12 * r:12 * (r + 1)]
            for j in range(NF):
                nc.tensor.matmul(xw3[:, j, :],
                                 xs8[g][:, r, P * j:P * (j + 1)], rhs,
                                 start=first, stop=last)
            nc.tensor.matmul(xw3[:, NF, :], ones128_8[:], rhs,
                             start=first, stop=last)


def _emit_ctx(nc, pools, xw, W3, tag, inv_scale=1.0):
    """xw/den -> xq8 -> diagonal-head G entries -> ctx [128, NF] f32 (SBUF)."""
    psG, ework = pools["psG"], pools["ework"]
    xw3 = xw[:].rearrange("p (a b) -> p a b", a=NF + 1)

    inv = ework.tile([P, 12], F32, tag=f"inv{tag}")
    nc.vector.tensor_scalar_add(inv[:], xw[:, NF * 12:NF * 12 + 12], 1e-8)
    nc.vector.reciprocal(inv[:], inv[:])
    if inv_scale != 1.0:
        nc.vector.tensor_scalar_mul(inv[:], inv[:], inv_scale)

    xq8 = ework.tile([P, NF * 12], F8, tag=f"xq{tag}")
    xq3 = xq8[:].rearrange("p (a b) -> p a b", a=NF)
    nc.vector.tensor_tensor(xq3, xw3[:, 0:NF, :],
                            inv[:, None, :].broadcast_to((P, NF, 12)),
                            mybir.AluOpType.mult)

    # only diagonal head pairs of G are needed: block m uses heads 2m, 2m+1
    gt = psG.tile([P, 2 * NF], F32, tag="g")
    for m in range(NF):
        for j in range(NF):
            nc.tensor.matmul(gt[:, 2 * m:2 * (m + 1)],
                             W3[:, j, P * m:P * (m + 1)],
                             xq3[:, j, 2 * m:2 * (m + 1)],
                             start=(j == 0), stop=(j == NF - 1))
    ctx = ework.tile([P, NF], F32, tag=f"ctx{tag}")
    gt3 = gt[:].rearrange("p (a b) -> p a b", a=NF)
    nc.vector.tensor_copy(ctx[0:64, :], gt3[0:64, :, 0])
    nc.vector.tensor_copy(ctx[64:P, :], gt3[64:P, :, 1])
    return ctx


def build_program(masked=False):
    nc = bacc.Bacc(trn_type="TRN2", target_bir_lowering=False)

    x8t_d = nc.dram_tensor("x8t", [P, NF * S], F8, kind="ExternalInput")
    xr8t_d = nc.dram_tensor("xr8t", [P, NF * S], F8, kind="ExternalInput")
    xs8_d = nc.dram_tensor("xs8", [P, NS * F], F8, kind="ExternalInput")
    aq8_d = nc.dram_tensor("aq8", [P, NF * 12], F8, kind="ExternalInput")
    wq8_d = nc.dram_tensor("wq8", [P, NF * F], F8, kind="ExternalInput")
    wk8_d = nc.dram_tensor("wk8", [P, NF * F], F8, kind="ExternalInput")
    wkt8_d = nc.dram_tensor("wkt8", [P, NF * F], F8, kind="ExternalInput")
    wqt8_d = nc.dram_tensor("wqt8", [P, NF * F], F8, kind="ExternalInput")
    wqr8_d = nc.dram_tensor("wqr8", [P, NF * F], F8, kind="ExternalInput")
    wka_d = nc.dram_tensor("wka", [P, NF * 12], F32, kind="ExternalInput")
    wobd_d = nc.dram_tensor("wobd", [P, P], F32, kind="ExternalInput")
    ones8_d = nc.dram_tensor("ones8", [P, P], F8, kind="ExternalInput")
    lm_d = nc.dram_tensor("lm", [P, NS], F32, kind="ExternalInput")
    out_d = nc.dram_tensor("out", [S, F], F16, kind="ExternalOutput")

    with tile.TileContext(nc) as tc:
        with ExitStack() as ctx:
            cpool = ctx.enter_context(tc.tile_pool(name="const", bufs=1))
            ework = ctx.enter_context(tc.tile_pool(name="ework", bufs=1))
            obuf = ctx.enter_context(tc.tile_pool(name="obuf", bufs=3))
            psW = ctx.enter_context(tc.tile_pool(name="psW", bufs=2, space="PSUM"))

            # ---- loads, in consumption order; small tensors first
            aq8 = cpool.tile([P, NF * 12], F8, tag="aq8")
            nc.sync.dma_start(aq8[:], aq8_d[:])
            ones128_8 = cpool.tile([P, P], F8, tag="ones8")
            nc.sync.dma_start(ones128_8[:], ones8_d[:])
            lm_sb = cpool.tile([P, NS], F32, tag="lm")
            if masked:
                nc.sync.dma_start(lm_sb[:], lm_d[:])
            wka = cpool.tile([P, NF * 12], F32, tag="wka")
            nc.sync.dma_start(wka[:], wka_d[:])
            wobd = cpool.tile([P, P], F32, tag="wobd")
            nc.sync.dma_start(wobd[:], wobd_d[:])

            x8t = cpool.tile([P, NF * S], F8, tag="x8t")
            x8t3 = x8t[:].rearrange("p (a b) -> p a b", a=NF)
            x8t_d3 = x8t_d[:].rearrange("p (a b) -> p a b", a=NF)
            xs8_tiles = []
            xs8 = []
            for g in range(NG):
                t = cpool.tile([P, GS * F], F8, tag=f"xs8_{g}")
                xs8_tiles.append(t)
                xs8.append(t[:].rearrange("p (a b) -> p a b", a=GS))
            # interleave: xT half 1, xs groups 0-3, xT half 2, xs groups 4-7
            nc.sync.dma_start(x8t3[:, :, 0:S // 2], x8t_d3[:, :, 0:S // 2])
            for g in range(NG // 2):
                nc.sync.dma_start(xs8_tiles[g][:],
                                  xs8_d[:, GS * F * g:GS * F * (g + 1)])
            nc.sync.dma_start(x8t3[:, :, S // 2:S], x8t_d3[:, :, S // 2:S])
            for g in range(NG // 2, NG):
                nc.sync.dma_start(xs8_tiles[g][:],
                                  xs8_d[:, GS * F * g:GS * F * (g + 1)])

            wq8 = cpool.tile([P, NF * F], F8, tag="wq8")
            nc.sync.dma_start(wq8[:], wq8_d[:])
            wkt8 = cpool.tile([P, NF * F], F8, tag="wkt8")
            nc.sync.dma_start(wkt8[:], wkt8_d[:])
            wk8 = cpool.tile([P, NF * F], F8, tag="wk8")
            nc.sync.dma_start(wk8[:], wk8_d[:])
            wqt8 = cpool.tile([P, NF * F], F8, tag="wqt8")
            nc.sync.dma_start(wqt8[:], wqt8_d[:])
            wqr8 = cpool.tile([P, NF * F], F8, tag="wqr8")
            nc.sync.dma_start(wqr8[:], wqr8_d[:])
            xr8t = cpool.tile([P, NF * S], F8, tag="xr8t")
            xr8t3 = xr8t[:].rearrange("p (a b) -> p a b", a=NF)
            xr8t_d3 = xr8t_d[:].rearrange("p (a b) -> p a b", a=NF)
            for q in range(4):
                lo, hi = S // 4 * q, S // 4 * (q + 1)
                nc.sync.dma_start(xr8t3[:, :, lo:hi], xr8t_d3[:, :, lo:hi])

            wq3 = wq8[:].rearrange("p (a b) -> p a b", a=NF)
            wk3 = wk8[:].rearrange("p (a b) -> p a b", a=NF)
            wkt3 = wkt8[:].rearrange("p (a b) -> p a b", a=NF)
            wqt3 = wqt8[:].rearrange("p (a b) -> p a b", a=NF)
            wqr3 = wqr8[:].rearrange("p (a b) -> p a b", a=NF)
            wka3 = wka[:].rearrange("p (a b) -> p a b", a=NF)
            aq3 = aq8[:].rearrange("p (a b) -> p a b", a=NF)
            cst = {"x8t3": x8t3, "xs8": xs8, "ones128_8": ones128_8,
                   "lm_sb": lm_sb}

            mr8 = ework.tile([P, NF * F], F8, tag="mr8")
            m8_3 = wq3
            mr8_3 = mr8[:].rearrange("p (a b) -> p a b", a=NF)

            with ExitStack() as pre:
                psS = pre.enter_context(tc.tile_pool(name="psS", bufs=2,
                                                     space="PSUM"))
                psXW = pre.enter_context(tc.tile_pool(name="psXW", bufs=1,
                                                      space="PSUM"))
                psG = pre.enter_context(tc.tile_pool(name="psG", bufs=1,
                                                     space="PSUM"))
                pools = {"psS": psS, "psXW": psXW, "psG": psG, "ework": ework}

                # ---- pass 1: query pooling + q_ctx (split by xT halves)
                e_q = []
                xw_q = psXW.tile([P, (NF + 1) * 12], F32, tag="xw")
                _emit_scores(nc, pools, cst, aq3, masked, 0, e_q)
                _emit_xw(nc, pools, cst, e_q, range(NG // 2), xw_q, 0, NS - 1)
                _emit_scores(nc, pools, cst, aq3, masked, 1, e_q)
                _emit_xw(nc, pools, cst, e_q, range(NG // 2, NG), xw_q,
                         0, NS - 1)
                qctx = _emit_ctx(nc, pools, xw_q, wq3, "q", inv_scale=1.0 / MS)

                # ---- A_k = Wk @ (q_ctx * Wka)
                g8 = ework.tile([P, NF * 12], F8, tag="g8")
                g3 = g8[:].rearrange("p (a b) -> p a b", a=NF)
                nc.vector.tensor_tensor(
                    g3, wka3, qctx[:, :, None].broadcast_to((P, NF, 12)),
                    mybir.AluOpType.mult)
                ak_ps = psG.tile([P, NF * 12], F32, tag="g")
                for ft in range(NF):
                    for fc in range(NF):
                        nc.tensor.matmul(ak_ps[:, 12 * ft:12 * (ft + 1)],
                                         wkt3[:, fc, P * ft:P * (ft + 1)],
                                         g3[:, fc, :],
                                         start=(fc == 0), stop=(fc == NF - 1))
                ak8 = ework.tile([P, NF * 12], F8, tag="ak8")
                nc.scalar.copy(ak8[:], ak_ps[:])
                ak3 = ak8[:].rearrange("p (a b) -> p a b", a=NF)

                # ---- pass 2: key pooling + k_ctx
                e_k = []
                xw_k = psXW.tile([P, (NF + 1) * 12], F32, tag="xw")
                _emit_scores(nc, pools, cst, ak3, masked, 0, e_k)
                _emit_scores(nc, pools, cst, ak3, masked, 1, e_k)
                _emit_xw(nc, pools, cst, e_k, range(NG), xw_k, 0, NS - 1)
                kc0 = _emit_ctx(nc, pools, xw_k, wk3, "k")
                kctx = ework.tile([P, NF], F32, tag="kctx")
                nc.vector.tensor_tensor(kctx[:], qctx[:], kc0[:],
                                        mybir.AluOpType.mult)

                # ---- M = Wq @ (blockdiag(kctx_h * Wo) + I), scaled by MS
                # r_all[:, j, :] = wobd (block-diag stacked Wo, x64) row-scaled
                # by kctx[:, j]; the +I (i.e. + Wq) lands via MS*I128 matmuls.
                r_all = ework.tile([P, NF * P], F16, tag="r_all")
                r3 = r_all[:].rearrange("p (a b) -> p a b", a=NF)
                nc.vector.tensor_tensor(
                    r3, wobd[:, None, :].broadcast_to((P, NF, P)),
                    kctx[:, :, None].broadcast_to((P, NF, P)),
                    mybir.AluOpType.mult)

                for ft in range(NF):
                    mc = psW.tile([P, F], F32, tag="wide")
                    for j in range(NF):
                        nc.tensor.matmul(mc[:, P * j:P * (j + 1)],
                                         wqt3[:, j, P * ft:P * (ft + 1)],
                                         r3[:, j, :], start=True, stop=True)
                    nc.vector.tensor_tensor(mr8_3[:, ft, :], mc[:],
                                            wqr3[:, ft, :],
                                            mybir.AluOpType.add)

            # ---- pass 3: out = (x8 + xr8) @ M8 + x8 @ Mr8, fp8 DoubleRow,
            # pair-major accumulation; two pools alternate -> 4 chunks in flight
            psT = ctx.enter_context(tc.tile_pool(name="psT", bufs=2,
                                                 space="PSUM"))
            for i in range(NS):
                ps = (psW if i % 2 == 0 else psT).tile([P, F], F32, tag="wide")
                n = 0
                for t in range(NF // 2):
                    for lhs3, rhs3 in ((x8t3, m8_3), (x8t3, mr8_3),
                                      (xr8t3, m8_3)):
                        for lo, hi in ((0, 512), (512, F)):
                            nc.tensor.matmul(
                                ps[:, lo:hi],
                                lhs3[:, 2 * t:2 * t + 2, P * i:P * (i + 1)],
                                rhs3[:, 2 * t:2 * t + 2, lo:hi],
                                start=(n == 0), stop=(n == 16),
                                perf_mode=DR)
                        n += 2
                ow = obuf.tile([P, F], F16, tag="ow")
                nc.scalar.mul(ow[:], ps[:], 1.0 / MS)
                nc.sync.dma_start(out_d[P * i:P * (i + 1), :], ow[:])

    nc.compile()
    return nc


def _get_program(masked=False):
    key = ("m" if masked else "u")
    if key not in _prog_cache:
        _prog_cache[key] = build_program(masked)
    return _prog_cache[key]


def _chunk_rows(a, np_dtype):
    """[R*128, C] -> [128, R*C] with chunk r of rows at cols [r*C:(r+1)*C]."""
    R = a.shape[0] // P
    return np.ascontiguousarray(
        a.reshape(R, P, a.shape[1]).transpose(1, 0, 2).reshape(P, -1)
    ).astype(np_dtype)


def _prep_weights(Wq, Wk, Wqa, Wka, Wo):
    Aq = (Wq @ Wqa).astype(np.float32)
    wobd = np.zeros((P, P), np.float32)
    wobd[0:64, 0:64] = MS * Wo
    wobd[64:P, 64:P] = MS * Wo
    return {
        "aq8": _chunk_rows(Aq, NP8),
        "wq8": _chunk_rows(MS * Wq, NP8),
        "wk8": _chunk_rows(Wk, NP8),
        "wkt8": _chunk_rows(np.ascontiguousarray(Wk.T), NP8),
        "wqt8": _chunk_rows(np.ascontiguousarray(Wq.T), NP8),
        "wqr8": _chunk_rows(
            MS * Wq - (MS * Wq).astype(NP8).astype(np.float32), NP8),
        "wka": _chunk_rows(Wka, np.float32),
        "wobd": wobd,
        "ones8": np.ones((P, P), NP8),
    }


def _prep_core_inputs(xb, maskb, w, masked):
    x8 = xb.astype(NP8)
    xr8 = (xb - x8.astype(np.float32)).astype(NP8)
    d = {
        "x8t": _chunk_rows(np.ascontiguousarray(x8.astype(np.float32).T), NP8),
        "xr8t": _chunk_rows(np.ascontiguousarray(xr8.astype(np.float32).T), NP8),
        "xs8": _chunk_rows(x8.astype(np.float32), NP8),
        "lm": np.zeros((P, NS), np.float32),
        **w,
    }
    if masked:
        lm = np.where(maskb > 0, 0.0, -60000.0).astype(np.float32) * ESC
        d["lm"] = np.ascontiguousarray(lm.reshape(NS, P).T)
    return d


def run(x, attn_mask, Wq, Wk, Wqa, Wka, Wo, trace=False):
    from concourse.bass_utils import run_bass_kernel_spmd

    masked = not bool(np.all(attn_mask == 1.0))
    nc = _get_program(masked)
    w = _prep_weights(Wq, Wk, Wqa, Wka, Wo)
    in_maps = [_prep_core_inputs(np.asarray(x[b]), np.asarray(attn_mask[b]),
                                 w, masked)
               for b in range(N_CORES)]
    res = run_bass_kernel_spmd(nc, in_maps, list(range(N_CORES)), trace=trace)
    out = np.stack([res.results[b]["out"].astype(np.float32)
                    for b in range(N_CORES)])
    return out, res


def kernel(x, attn_mask, Wq, Wk, Wqa, Wka, Wo):
    out, _ = run(np.asarray(x, dtype=np.float32),
                 np.asarray(attn_mask, dtype=np.float32),
                 np.asarray(Wq, dtype=np.float32),
                 np.asarray(Wk, dtype=np.float32),
                 np.asarray(Wqa, dtype=np.float32),
                 np.asarray(Wka, dtype=np.float32),
                 np.asarray(Wo, dtype=np.float32))
    return out
